# revision 11
# baseline (speedup 1.0000x reference)
"""Multi-head attention (B=4,S=2048,D=1024,H=16,dh=65) on 8 TRN2 NeuronCores.

Sharding: batch x head-half. Core c handles batch c//2 and heads
(c%2)*8..(c%2)*8+8 (P-slice of 520). Each core computes its QKV projections,
attention, and a partial out-projection; the host sums the two partials per
batch and adds bo.

v2 layout: Q/K projections are computed directly in transposed [dh, s] form
(per-head weight slice stationary, x moving with N=2048 streams) which
removes all PE transposes and PSUM-shuffle copies of v1; V stays s-major
(x stationary) for the attention AV matmul. Softmax runs unnormalized in
bf16 with the row-sum harvested from a trailing ones-column in V.
"""

import math
import sys
from collections import deque
from contextlib import ExitStack

import numpy as np
import ml_dtypes

sys.path.insert(0, "/opt/trn_rl_repo")

import concourse.bass as bass
import concourse.mybir as mybir
import concourse.tile as tile_mod
from concourse.bass_utils import run_bass_kernel_spmd
from concourse.vector_clock import ScopedClock

# ---------------------------------------------------------------------------
# Patch for this container's walrus build: it rejects instructions carrying
# more than one semaphore wait ("Too many sync wait commands"), but Tile's
# wait assigner freely attaches several. Split excess waits onto bass_nofuse
# InstNoOp carriers on the same engine, committed immediately before the
# instruction (same-engine program order => over-synchronization only).
# ---------------------------------------------------------------------------
_MAX_WAITS = 1

_orig_commit = tile_mod.TileContext._commit_instruction


def _split_waits(self, inst, commit):
    si = inst.sync_info
    if si is None or len(si.on_wait) <= _MAX_WAITS:
        return
    waits = list(si.on_wait)
    sem_w = [w for w in waits if getattr(w, "sync_type", "semaphore") == "semaphore"]
    other_w = [w for w in waits if getattr(w, "sync_type", "semaphore") != "semaphore"]
    keep_budget = _MAX_WAITS - len(other_w)
    if keep_budget < 0:
        return
    keep = other_w + (sem_w[-keep_budget:] if keep_budget > 0 else [])
    excess = sem_w[: len(sem_w) - max(keep_budget, 0)]
    if not excess:
        return
    for i, w in enumerate(excess):
        nop = mybir.InstNoOp(
            name=f"{inst.name}-sw{i}",
            sync_info=mybir.SyncInfo(on_wait=[w], on_update=[]),
            bass_nofuse=True,
            engine=inst.engine,
        )
        commit(nop)
    inst.sync_info = mybir.SyncInfo(on_wait=keep, on_update=list(si.on_update))


def _patched_commit(self, inst, lazy_reg_writes: bool = True):
    if inst.engine != mybir.EngineType.Unassigned:
        _split_waits(self, inst, lambda n: _orig_commit(self, n, False))
    return _orig_commit(self, inst, lazy_reg_writes)


def _patched_drain_and_barrier(self, tick_clock, wait_clock):
    drain_inst = self.nc.sync.drain()
    wait_clock.add_sem_waits(
        drain_inst.ins, ScopedClock({None: tick_clock.global_clock})
    )
    si = drain_inst.ins.sync_info
    if si is not None and len(si.on_wait) > _MAX_WAITS:
        waits = list(si.on_wait)
        drain_inst.ins.sync_info = mybir.SyncInfo(
            on_wait=waits[:_MAX_WAITS], on_update=list(si.on_update)
        )
        for w in waits[_MAX_WAITS:]:
            n = self.nc.sync.nop(nofuse=True)
            n.ins.sync_info = mybir.SyncInfo(on_wait=[w], on_update=[])
    self.nc.all_engine_barrier()
    popped = self.nc._tile_sem_poison_stack.pop()
    assert popped is self._sem_poison
    self.nc.clear_and_free_semaphores(list(self.sems.allocated().values()))
    self.nc.all_engine_barrier()


tile_mod.TileContext._commit_instruction = _patched_commit
tile_mod.TileContext._drain_and_barrier = _patched_drain_and_barrier

# ---------------------------------------------------------------------------

B, S, D, H = 4, 2048, 1024, 16
DH = D // H + 1          # 65
P = H * DH               # 1040
HPC = H // 2             # heads per core
PC = HPC * DH            # 520, per-core P slice
N_CORES = 8

MT = S // 128            # 16 row blocks / k tiles
KT = 16                  # k tiles per attention
QB = 4                   # q blocks of 512
QW = 512
RKT = 2                  # k-tiles per score round (2 banks, double-buffered)
NR = KT // RKT           # 8 rounds

F32 = mybir.dt.float32
BF16 = mybir.dt.bfloat16
BF = ml_dtypes.bfloat16

_BUILT = {}


def _build_nc():
    nc = bass.Bass("TRN2", target_bir_lowering=False, debug=False,
                   num_devices=N_CORES)

    xq_d = nc.dram_tensor("xq", [D, S], BF16, kind="ExternalInput").ap()
    xk_d = nc.dram_tensor("xk", [D, S], BF16, kind="ExternalInput").ap()
    xv_d = nc.dram_tensor("xv", [D, S], BF16, kind="ExternalInput").ap()
    # maskH[qb, p, j*QW+q] = maskT[j*128+p, qb*512+q] (multiplicative 0/1)
    mh = nc.dram_tensor("maskH", [QB, 128, KT * QW], BF16,
                        kind="ExternalInput").ap()
    wq_d = nc.dram_tensor("wqT", [D, PC], BF16, kind="ExternalInput").ap()
    wk_d = nc.dram_tensor("wkT", [D, PC], BF16, kind="ExternalInput").ap()
    wv_d = nc.dram_tensor("wvT", [D, PC], BF16, kind="ExternalInput").ap()
    bqT_d = nc.dram_tensor("bqT", [DH, HPC], F32, kind="ExternalInput").ap()
    bkT_d = nc.dram_tensor("bkT", [DH, HPC], F32, kind="ExternalInput").ap()
    bv_d = nc.dram_tensor("bv", [1, PC], BF16, kind="ExternalInput").ap()
    wo_d = nc.dram_tensor("woT", [PC, D], BF16, kind="ExternalInput").ap()
    sel8_d = nc.dram_tensor("sel8", [HPC, HPC * DH], F32,
                            kind="ExternalInput").ap()
    out = nc.dram_tensor("out", [S, D], F32, kind="ExternalOutput").ap()

    # packed 128-row k-tile ranges of the 520-row concatT / WoT
    PKT = [(0, 128), (128, 256), (256, 384), (384, 512), (512, 520)]

    with tile_mod.TileContext(nc) as tc:
        with tc.tile_pool(name="const", bufs=1) as pconst, \
             tc.tile_pool(name="qkT", bufs=1) as pqkT, \
             tc.tile_pool(name="vh", bufs=MT + 1) as pvh:

            ones_col = pconst.tile([1, 128], BF16, tag="ones")
            nc.gpsimd.memset(ones_col[:], 1.0)
            sel8 = pconst.tile([HPC, HPC * DH], F32, tag="sel8")
            nc.sync.dma_start(sel8[:], sel8_d[:])

            # [65, proj(q=0,k=1), head, S]
            qkT = pqkT.tile([DH, 2, HPC, S], BF16, tag="qkT")
            # v k-tiles with trailing ones column: [128, head, 65+1]
            vh = [pvh.tile([128, HPC, DH + 1], BF16, tag="vh", name=f"vh{j}")
                  for j in range(MT)]
            for j in range(MT):
                nc.gpsimd.memset(vh[j][:, :, DH:DH + 1], 1.0)

            # --------------- phase 1a: V projection (s-major) --------------
            _xw_stack = ExitStack()
            px = _xw_stack.enter_context(tc.tile_pool(name="px", bufs=16))
            pw = _xw_stack.enter_context(tc.tile_pool(name="pw", bufs=16))
            pb = _xw_stack.enter_context(tc.tile_pool(name="pb", bufs=1))

            def load_x(xd, tag):
                # chunk-major loads so early row-blocks of every d-tile land
                # first and compute can start ~3us in
                xts = [px.tile([128, S], BF16, tag="x", name=f"x{tag}{d}")
                       for d in range(8)]
                for c in range(4):
                    for d in range(8):
                        nc.sync.dma_start(
                            xts[d][:, c * QW:(c + 1) * QW],
                            xd[d * 128:(d + 1) * 128, c * QW:(c + 1) * QW])
                return xts

            def load_w(wd, tag):
                wts = []
                for d in range(8):
                    wt = pw.tile([128, HPC, DH], BF16, tag="w",
                                 name=f"w{tag}{d}")
                    nc.sync.dma_start(wt[:], wd[d * 128:(d + 1) * 128, :])
                    wts.append(wt)
                return wts

            xv_t = load_x(xv_d, "v")
            wv_t = load_w(wv_d, "v")
            bv_t = pb.tile([1, HPC, DH], BF16, tag="bv")
            nc.sync.dma_start(bv_t[:], bv_d[:])
            # K/Q operands: start their DMAs now so they land during V math
            xk_t = load_x(xk_d, "k")
            wk_t = load_w(wk_d, "k")
            bkT_t = pb.tile([DH, HPC], F32, tag="bkT")
            nc.sync.dma_start(bkT_t[:], bkT_d[:])
            bqT_t = pb.tile([DH, HPC], F32, tag="bqT")
            nc.sync.dma_start(bqT_t[:], bqT_d[:])

            with tc.tile_pool(name="psV", bufs=4, space="PSUM") as psV:
                for m in range(MT):
                    pss = [psV.tile([128, 4, DH], F32, tag="psV",
                                    name=f"psv{m}_{hf}") for hf in range(2)]
                    for hf in range(2):
                        nc.tensor.matmul(pss[hf][:], ones_col[0:1, :],
                                         bv_t[0:1, hf * 4:hf * 4 + 4, :],
                                         start=True, stop=False)
                    for d in range(8):
                        for hf in range(2):
                            nc.tensor.matmul(
                                pss[hf][:],
                                xv_t[d][:, m * 128:(m + 1) * 128],
                                wv_t[d][:, hf * 4:hf * 4 + 4, :],
                                start=False, stop=(d == 7))
                    for hf in range(2):
                        nc.vector.tensor_copy(
                            vh[m][:, hf * 4:hf * 4 + 4, 0:DH], pss[hf][:])

            # xq/wq reuse the V buffers (auto-synced on last V-proj reader)
            xq_t = load_x(xq_d, "q")
            wq_t = load_w(wq_d, "q")

            # --------- phase 1b: K and Q projections, direct [dh, s] -------
            with tc.tile_pool(name="psKQ", bufs=2, space="PSUM") as psKQ:
                for h in range(HPC):
                    for pi, wt, xt, bt in ((1, wk_t, xk_t, bkT_t),
                                           (0, wq_t, xq_t, bqT_t)):
                        ps = psKQ.tile([DH, 4, QW], F32, tag="psKQ",
                                       name=f"pskq{pi}_{h}")
                        for d in range(8):
                            for c in range(4):
                                nc.tensor.matmul(
                                    ps[:, c, :],
                                    wt[d][:, h, :],
                                    xt[d][:, c * QW:(c + 1) * QW],
                                    start=(d == 0), stop=(d == 7))
                        for c in range(4):
                            nc.vector.tensor_scalar_add(
                                qkT[0:DH, pi, h, c * QW:(c + 1) * QW],
                                ps[:, c, :], bt[:, h:h + 1])
            _xw_stack.close()   # free x/w pools before attention phase

            # ---------------- phase 2+3: attention + out-proj --------------
            with tc.tile_pool(name="pm", bufs=3) as pm, \
                 tc.tile_pool(name="pp", bufs=4) as pp, \
                 tc.tile_pool(name="pc", bufs=1) as pc, \
                 tc.tile_pool(name="pwo", bufs=1) as pwo, \
                 tc.tile_pool(name="po", bufs=2) as po, \
                 tc.tile_pool(name="pt2", bufs=4) as pt2, \
                 tc.tile_pool(name="psS", bufs=2, space="PSUM") as psS, \
                 tc.tile_pool(name="psA", bufs=4, space="PSUM") as psA:

                # packed concatT: 128-row tiles covering rows 0..520
                ccp = [pc.tile([b - a, S], BF16, tag=f"ccp{i}",
                               name=f"ccp{i}")
                       for i, (a, b) in enumerate(PKT)]
                wop = []
                for i, (a, b) in enumerate(PKT):
                    w = pwo.tile([b - a, D], BF16, tag=f"wop{i}",
                                 name=f"wop{i}")
                    nc.sync.dma_start(w[:], wo_d[a:b, :])
                    wop.append(w)

                inv_sqrt = 1.0 / math.sqrt(float(DH))
                state = {}
                # AV rounds lag the score/exp stream by AV_LAG slots so the
                # next head's scores issue before this head's tail AV matmuls
                # (keeps the scalar engine's exp stream gapless); bgq drips
                # norm/out-proj PE work one small closure per round slot.
                pending = deque()
                bgq = deque()
                AV_LAG = 2

                def drain_slot():
                    if len(pending) > AV_LAG:
                        pending.popleft()()
                    if bgq:
                        bgq.popleft()()

                def attn_head(qb, h, mts, rsall, uovs):
                    ctx = {}
                    for r in range(NR):
                        ss = psS.tile([128, RKT, QW], F32, tag="psS",
                                      name=f"ss{qb}_{h}_{r}")
                        for jj in range(RKT):
                            j = r * RKT + jj
                            nc.tensor.matmul(
                                ss[:, jj, :],
                                qkT[0:DH, 1, h, j * 128:(j + 1) * 128],
                                qkT[0:DH, 0, h, qb * QW:(qb + 1) * QW],
                                start=True, stop=True)
                        pt = pp.tile([128, RKT, QW], BF16, tag="pT",
                                     name=f"pt{qb}_{h}_{r}")
                        nc.scalar.activation(
                            pt[:], ss[:],
                            mybir.ActivationFunctionType.Exp,
                            scale=inv_sqrt)
                        mt = mts[r // (NR // 2)]
                        rr = r % (NR // 2)
                        nc.vector.tensor_mul(
                            pt[:], pt[:], mt[:, rr * RKT:(rr + 1) * RKT, :])

                        def av(r=r, pt=pt, qb=qb, h=h, ctx=ctx):
                            if "ov" not in ctx:
                                ctx["ov"] = psA.tile(
                                    [128, QW], F32, tag="psA",
                                    name=f"ov{qb}_{h}")
                            ov = ctx["ov"]
                            for jj in range(RKT):
                                j = r * RKT + jj
                                nc.tensor.matmul(
                                    ov[0:DH + 1, :], vh[j][:, h, :],
                                    pt[:, jj, :],
                                    start=(j == 0), stop=(j == KT - 1))
                        pending.append(av)
                        drain_slot()

                    def evac(qb=qb, h=h, ctx=ctx, rsall=rsall, uovs=uovs):
                        ov = ctx["ov"]
                        uov = pt2.tile([DH, QW], BF16, tag="uov",
                                       name=f"uov{qb}_{h}", bufs=2 * HPC + 1)
                        nc.vector.tensor_copy(uov[:], ov[0:DH, :])
                        rs2 = pt2.tile([66, QW], F32, tag="rs2",
                                       name=f"rs2_{qb}_{h}", bufs=2)
                        nc.vector.tensor_copy(rs2[64:66, :], ov[64:66, :])
                        nc.gpsimd.dma_start(rsall[h:h + 1, :], rs2[65:66, :])
                        uovs.append(uov)
                    pending.append(evac)
                    # heads push 9 closures over 8 round slots; one extra pop
                    # here keeps the queue depth (and the AV lag) constant
                    if len(pending) > AV_LAG:
                        pending.popleft()()

                def norm_head_cl(qb, h):
                    def cl():
                        st = state[qb]
                        if "rcall" not in st:
                            rcall = pt2.tile([HPC, QW], F32, tag="rcall",
                                             name=f"rcall{qb}", bufs=2)
                            nc.vector.reciprocal(rcall[:], st["rsall"][:])
                            st["rcall"] = rcall
                        rbp = psA.tile([128, QW], F32, tag="psA",
                                       name=f"rbp{qb}_{h}")
                        nc.tensor.matmul(rbp[0:DH, :],
                                         sel8[:, h * DH:(h + 1) * DH],
                                         st["rcall"][:],
                                         start=True, stop=True)
                        cch = pt2.tile([DH, QW], BF16, tag="cch",
                                       name=f"cch{qb}_{h}", bufs=2)
                        nc.vector.tensor_mul(cch[:], rbp[0:DH, :],
                                             st["uovs"][h][:])
                        # pack into 128-row concatT tiles (DMA shifts rows)
                        r0 = h * DH
                        for i, (a, b) in enumerate(PKT):
                            lo, hi = max(r0, a), min(r0 + DH, b)
                            if lo < hi:
                                nc.gpsimd.dma_start(
                                    ccp[i][lo - a:hi - a,
                                           qb * QW:(qb + 1) * QW],
                                    cch[lo - r0:hi - r0, :])
                    return cl

                def outproj_cls(m):
                    ctx = {}

                    def mk_mm(n, i, a, b):
                        def cl():
                            if n not in ctx:
                                ctx[n] = psA.tile([128, QW], F32, tag="psA",
                                                  name=f"psop{m}_{n}")
                            nc.tensor.matmul(
                                ctx[n][:], ccp[i][:, m * 128:(m + 1) * 128],
                                wop[i][:, n * QW:(n + 1) * QW],
                                start=(i == 0), stop=(i == len(PKT) - 1))
                        return cl

                    def mk_copy(n):
                        def cl():
                            if "osb" not in ctx:
                                ctx["osb"] = po.tile([128, D], F32, tag="osb",
                                                     name=f"osb{m}")
                            nc.vector.tensor_copy(
                                ctx["osb"][:, n * QW:(n + 1) * QW],
                                ctx[n][:])
                        return cl

                    def mk_dma():
                        def cl():
                            nc.sync.dma_start(
                                out[m * 128:(m + 1) * 128, :], ctx["osb"][:])
                        return cl

                    cls = []
                    for n in range(2):
                        for i, (a, b) in enumerate(PKT):
                            cls.append(mk_mm(n, i, a, b))
                        cls.append(mk_copy(n))
                    cls.append(mk_dma())
                    return cls

                for qb in range(QB):
                    mts = []
                    for hf in range(2):
                        mt = pm.tile([128, KT // 2, QW], BF16, tag="mask",
                                     name=f"mask{qb}_{hf}")
                        nc.sync.dma_start(
                            mt[:], mh[qb, :, hf * (KT // 2) * QW:
                                      (hf + 1) * (KT // 2) * QW])
                        mts.append(mt)
                    rsall = pt2.tile([HPC, QW], F32, tag="rsall",
                                     name=f"rsall{qb}", bufs=2)
                    uovs = []
                    state[qb] = {"rsall": rsall, "uovs": uovs}
                    for h in range(HPC):
                        attn_head(qb, h, mts, rsall, uovs)
                        if qb > 0 and h == 0:
                            # safe now: evac(qb-1, h7) popped during h0,
                            # so rsall/uovs of qb-1 are fully emitted
                            for hh in range(HPC):
                                bgq.append(norm_head_cl(qb - 1, hh))
                            for m in range((qb - 1) * 4, qb * 4):
                                bgq.extend(outproj_cls(m))
                while pending:
                    pending.popleft()()
                for h in range(HPC):
                    bgq.append(norm_head_cl(QB - 1, h))
                for m in range((QB - 1) * 4, QB * 4):
                    bgq.extend(outproj_cls(m))
                while bgq:
                    bgq.popleft()()

    return nc


def _prep_inputs(q, k, v, mask, Wq, bqv, Wk, bkv, Wv, bvv, Wo):
    """Per-core input maps (numpy, host-side shard + cast)."""
    in_maps = []
    sel8 = np.zeros((HPC, HPC * DH), np.float32)
    for h in range(HPC):
        sel8[h, h * DH:(h + 1) * DH] = 1.0
    mask_h = {}
    for b in range(B):
        mt = (mask[b, 0] != 0).astype(np.float32).T  # [k, q]
        m4 = mt.reshape(KT, 128, QB, QW).transpose(2, 1, 0, 3)
        mask_h[b] = np.ascontiguousarray(m4.reshape(QB, 128, KT * QW)).astype(BF)
    for c in range(N_CORES):
        b, hh = c // 2, c % 2
        sl = slice(hh * PC, (hh + 1) * PC)
        in_maps.append({
            "xq": np.ascontiguousarray(q[b].T).astype(BF),
            "xk": np.ascontiguousarray(k[b].T).astype(BF),
            "xv": np.ascontiguousarray(v[b].T).astype(BF),
            "maskH": mask_h[b],
            "wqT": np.ascontiguousarray(Wq[sl, :].T).astype(BF),
            "wkT": np.ascontiguousarray(Wk[sl, :].T).astype(BF),
            "wvT": np.ascontiguousarray(Wv[sl, :].T).astype(BF),
            "bqT": np.ascontiguousarray(
                bqv[sl].reshape(HPC, DH).T).astype(np.float32),
            "bkT": np.ascontiguousarray(
                bkv[sl].reshape(HPC, DH).T).astype(np.float32),
            "bv": bvv[sl].reshape(1, PC).astype(BF),
            "woT": np.ascontiguousarray(Wo[:, sl].T).astype(BF),
            "sel8": sel8,
        })
    return in_maps


def run_sharded(in_maps, **kwargs):
    if "nc" not in _BUILT:
        _BUILT["nc"] = _build_nc()
    return run_bass_kernel_spmd(_BUILT["nc"], in_maps,
                                core_ids=list(range(N_CORES)), **kwargs)


def kernel(q, k, v, mask, Wq, bq, Wk, bk, Wv, bv, Wo, bo):
    q = np.asarray(q, np.float32)
    k = np.asarray(k, np.float32)
    v = np.asarray(v, np.float32)
    mask = np.asarray(mask)
    in_maps = _prep_inputs(q, k, v, mask,
                           np.asarray(Wq, np.float32), np.asarray(bq, np.float32),
                           np.asarray(Wk, np.float32), np.asarray(bk, np.float32),
                           np.asarray(Wv, np.float32), np.asarray(bv, np.float32),
                           np.asarray(Wo, np.float32))
    res = run_sharded(in_maps)
    bo32 = np.asarray(bo, np.float32)
    out = np.empty((B, S, D), np.float32)
    for b in range(B):
        out[b] = res.results[2 * b]["out"] + res.results[2 * b + 1]["out"] + bo32
    return out


# revision 18
# speedup vs baseline: 1.0579x; 1.0579x over previous
"""Multi-head attention (B=4,S=2048,D=1024,H=16,dh=65) on 8 TRN2 NeuronCores.

Sharding: batch x head-half. Core c handles batch c//2 and heads
(c%2)*8..(c%2)*8+8 (P-slice of 520). Each core computes its QKV projections,
attention, and a partial out-projection; the host sums the two partials per
batch and adds bo.

v2 layout: Q/K projections are computed directly in transposed [dh, s] form
(per-head weight slice stationary, x moving with N=2048 streams) which
removes all PE transposes and PSUM-shuffle copies of v1; V stays s-major
(x stationary) for the attention AV matmul. Softmax runs unnormalized in
bf16 with the row-sum harvested from a trailing ones-column in V.
"""

import math
import sys
from collections import deque
from contextlib import ExitStack

import numpy as np
import ml_dtypes

sys.path.insert(0, "/opt/trn_rl_repo")

import concourse.bass as bass
import concourse.mybir as mybir
import concourse.tile as tile_mod
from concourse.bass_utils import run_bass_kernel_spmd
from concourse.vector_clock import ScopedClock

# ---------------------------------------------------------------------------
# Patch for this container's walrus build: it rejects instructions carrying
# more than one semaphore wait ("Too many sync wait commands"), but Tile's
# wait assigner freely attaches several. Split excess waits onto bass_nofuse
# InstNoOp carriers on the same engine, committed immediately before the
# instruction (same-engine program order => over-synchronization only).
# ---------------------------------------------------------------------------
_MAX_WAITS = 1

_orig_commit = tile_mod.TileContext._commit_instruction


def _split_waits(self, inst, commit):
    si = inst.sync_info
    if si is None or len(si.on_wait) <= _MAX_WAITS:
        return
    waits = list(si.on_wait)
    sem_w = [w for w in waits if getattr(w, "sync_type", "semaphore") == "semaphore"]
    other_w = [w for w in waits if getattr(w, "sync_type", "semaphore") != "semaphore"]
    keep_budget = _MAX_WAITS - len(other_w)
    if keep_budget < 0:
        return
    keep = other_w + (sem_w[-keep_budget:] if keep_budget > 0 else [])
    excess = sem_w[: len(sem_w) - max(keep_budget, 0)]
    if not excess:
        return
    for i, w in enumerate(excess):
        nop = mybir.InstNoOp(
            name=f"{inst.name}-sw{i}",
            sync_info=mybir.SyncInfo(on_wait=[w], on_update=[]),
            bass_nofuse=True,
            engine=inst.engine,
        )
        commit(nop)
    inst.sync_info = mybir.SyncInfo(on_wait=keep, on_update=list(si.on_update))


def _patched_commit(self, inst, lazy_reg_writes: bool = True):
    if inst.engine != mybir.EngineType.Unassigned:
        _split_waits(self, inst, lambda n: _orig_commit(self, n, False))
    return _orig_commit(self, inst, lazy_reg_writes)


def _patched_drain_and_barrier(self, tick_clock, wait_clock):
    drain_inst = self.nc.sync.drain()
    wait_clock.add_sem_waits(
        drain_inst.ins, ScopedClock({None: tick_clock.global_clock})
    )
    si = drain_inst.ins.sync_info
    if si is not None and len(si.on_wait) > _MAX_WAITS:
        waits = list(si.on_wait)
        drain_inst.ins.sync_info = mybir.SyncInfo(
            on_wait=waits[:_MAX_WAITS], on_update=list(si.on_update)
        )
        for w in waits[_MAX_WAITS:]:
            n = self.nc.sync.nop(nofuse=True)
            n.ins.sync_info = mybir.SyncInfo(on_wait=[w], on_update=[])
    self.nc.all_engine_barrier()
    popped = self.nc._tile_sem_poison_stack.pop()
    assert popped is self._sem_poison
    self.nc.clear_and_free_semaphores(list(self.sems.allocated().values()))
    self.nc.all_engine_barrier()


tile_mod.TileContext._commit_instruction = _patched_commit
tile_mod.TileContext._drain_and_barrier = _patched_drain_and_barrier

# ---------------------------------------------------------------------------

B, S, D, H = 4, 2048, 1024, 16
DH = D // H + 1          # 65
P = H * DH               # 1040
HPC = H // 2             # heads per core
PC = HPC * DH            # 520, per-core P slice
N_CORES = 8

MT = S // 128            # 16 row blocks / k tiles
KT = 16                  # k tiles per attention
QB = 4                   # q blocks of 512
QW = 512
RKT = 2                  # k-tiles per score round (2 banks, double-buffered)
NR = KT // RKT           # 8 rounds

F32 = mybir.dt.float32
BF16 = mybir.dt.bfloat16
BF = ml_dtypes.bfloat16

_BUILT = {}


def _build_nc():
    nc = bass.Bass("TRN2", target_bir_lowering=False, debug=False,
                   num_devices=N_CORES)

    xq_d = nc.dram_tensor("xq", [D, S], BF16, kind="ExternalInput").ap()
    xk_d = nc.dram_tensor("xk", [D, S], BF16, kind="ExternalInput").ap()
    xv_d = nc.dram_tensor("xv", [D, S], BF16, kind="ExternalInput").ap()
    # maskH[qb, p, j*QW+q] = maskT[j*128+p, qb*512+q] (multiplicative 0/1)
    mh = nc.dram_tensor("maskH", [QB, 128, KT * QW], BF16,
                        kind="ExternalInput").ap()
    wq_d = nc.dram_tensor("wqT", [D, PC], BF16, kind="ExternalInput").ap()
    wk_d = nc.dram_tensor("wkT", [D, PC], BF16, kind="ExternalInput").ap()
    wv_d = nc.dram_tensor("wvT", [D, PC], BF16, kind="ExternalInput").ap()
    bqT_d = nc.dram_tensor("bqT", [DH, HPC], F32, kind="ExternalInput").ap()
    bkT_d = nc.dram_tensor("bkT", [DH, HPC], F32, kind="ExternalInput").ap()
    bv_d = nc.dram_tensor("bv", [1, PC], BF16, kind="ExternalInput").ap()
    wo_d = nc.dram_tensor("woT", [PC, D], BF16, kind="ExternalInput").ap()
    sel8_d = nc.dram_tensor("sel8", [HPC, HPC * DH], BF16,
                            kind="ExternalInput").ap()
    out = nc.dram_tensor("out", [S, D], F32, kind="ExternalOutput").ap()

    # packed 128-row k-tile ranges of the 520-row concatT / WoT
    PKT = [(0, 128), (128, 256), (256, 384), (384, 512), (512, 520)]

    with tile_mod.TileContext(nc) as tc:
        with tc.tile_pool(name="const", bufs=1) as pconst, \
             tc.tile_pool(name="qkT", bufs=1) as pqkT, \
             tc.tile_pool(name="vh", bufs=MT + 1) as pvh:

            ones_col = pconst.tile([1, 128], BF16, tag="ones")
            nc.gpsimd.memset(ones_col[:], 1.0)
            sel8 = pconst.tile([HPC, HPC * DH], BF16, tag="sel8")
            nc.sync.dma_start(sel8[:], sel8_d[:])

            # [65, proj(q=0,k=1), head, S]
            qkT = pqkT.tile([DH, 2, HPC, S], BF16, tag="qkT")
            # v k-tiles with trailing ones column: [128, head, 65+1]
            vh = [pvh.tile([128, HPC, DH + 1], BF16, tag="vh", name=f"vh{j}")
                  for j in range(MT)]
            for j in range(MT):
                nc.gpsimd.memset(vh[j][:, :, DH:DH + 1], 1.0)

            # --------------- phase 1a: V projection (s-major) --------------
            _xw_stack = ExitStack()
            px = _xw_stack.enter_context(tc.tile_pool(name="px", bufs=16))
            pw = _xw_stack.enter_context(tc.tile_pool(name="pw", bufs=16))
            pb = _xw_stack.enter_context(tc.tile_pool(name="pb", bufs=1))

            def load_x(xd, tag):
                # full-tile DMAs: each dma_start costs ~600ns of queue-issue
                # time regardless of size, so fewer+bigger wins
                xts = []
                for d in range(8):
                    xt = px.tile([128, S], BF16, tag="x", name=f"x{tag}{d}")
                    nc.sync.dma_start(xt[:], xd[d * 128:(d + 1) * 128, :])
                    xts.append(xt)
                return xts

            def load_w(wd, tag):
                # weights ride the gpsimd DMA queue so they are not stuck
                # behind the big x transfers on the sync queue
                wts = []
                for d in range(8):
                    wt = pw.tile([128, HPC, DH], BF16, tag="w",
                                 name=f"w{tag}{d}")
                    nc.gpsimd.dma_start(wt[:], wd[d * 128:(d + 1) * 128, :])
                    wts.append(wt)
                return wts

            wv_t = load_w(wv_d, "v")
            bv_t = pb.tile([1, HPC, DH], BF16, tag="bv")
            nc.gpsimd.dma_start(bv_t[:], bv_d[:])
            wk_t = load_w(wk_d, "k")
            bkT_t = pb.tile([DH, HPC], F32, tag="bkT")
            nc.gpsimd.dma_start(bkT_t[:], bkT_d[:])
            bqT_t = pb.tile([DH, HPC], F32, tag="bqT")
            nc.gpsimd.dma_start(bqT_t[:], bqT_d[:])
            xv_t = load_x(xv_d, "v")
            # K/Q inputs: start their DMAs now so they land during V math
            xk_t = load_x(xk_d, "k")

            with tc.tile_pool(name="psV", bufs=4, space="PSUM") as psV:
                for m in range(MT):
                    pss = [psV.tile([128, 4, DH], F32, tag="psV",
                                    name=f"psv{m}_{hf}") for hf in range(2)]
                    for hf in range(2):
                        nc.tensor.matmul(pss[hf][:], ones_col[0:1, :],
                                         bv_t[0:1, hf * 4:hf * 4 + 4, :],
                                         start=True, stop=False)
                    for d in range(8):
                        for hf in range(2):
                            nc.tensor.matmul(
                                pss[hf][:],
                                xv_t[d][:, m * 128:(m + 1) * 128],
                                wv_t[d][:, hf * 4:hf * 4 + 4, :],
                                start=False, stop=(d == 7))
                    for hf in range(2):
                        nc.vector.tensor_copy(
                            vh[m][:, hf * 4:hf * 4 + 4, 0:DH], pss[hf][:])

            # xq/wq reuse the V buffers (auto-synced on last V-proj reader)
            wq_t = load_w(wq_d, "q")
            xq_t = load_x(xq_d, "q")

            # --------- phase 1b: K and Q projections, direct [dh, s] -------
            with tc.tile_pool(name="psKQ", bufs=2, space="PSUM") as psKQ:
                for pi, wt, xt, bt in ((1, wk_t, xk_t, bkT_t),
                                       (0, wq_t, xq_t, bqT_t)):
                    for h in range(HPC):
                        ps = psKQ.tile([DH, 4, QW], F32, tag="psKQ",
                                       name=f"pskq{pi}_{h}")
                        for d in range(8):
                            for c in range(4):
                                nc.tensor.matmul(
                                    ps[:, c, :],
                                    wt[d][:, h, :],
                                    xt[d][:, c * QW:(c + 1) * QW],
                                    start=(d == 0), stop=(d == 7))
                        for c in range(4):
                            nc.vector.tensor_scalar_add(
                                qkT[0:DH, pi, h, c * QW:(c + 1) * QW],
                                ps[:, c, :], bt[:, h:h + 1])
            _xw_stack.close()   # free x/w pools before attention phase

            # ---------------- phase 2+3: attention + out-proj --------------
            with tc.tile_pool(name="pm", bufs=3) as pm, \
                 tc.tile_pool(name="pp", bufs=4) as pp, \
                 tc.tile_pool(name="pc", bufs=1) as pc, \
                 tc.tile_pool(name="pwo", bufs=1) as pwo, \
                 tc.tile_pool(name="po", bufs=2) as po, \
                 tc.tile_pool(name="pt2", bufs=4) as pt2, \
                 tc.tile_pool(name="psS", bufs=2, space="PSUM") as psS, \
                 tc.tile_pool(name="psA", bufs=4, space="PSUM") as psA:

                # packed concatT: 128-row tiles covering rows 0..520
                ccp = [pc.tile([b - a, S], BF16, tag=f"ccp{i}",
                               name=f"ccp{i}")
                       for i, (a, b) in enumerate(PKT)]
                wop = []
                for i, (a, b) in enumerate(PKT):
                    w = pwo.tile([b - a, D], BF16, tag=f"wop{i}",
                                 name=f"wop{i}")
                    nc.sync.dma_start(w[:], wo_d[a:b, :])
                    wop.append(w)

                inv_sqrt = 1.0 / math.sqrt(float(DH))
                state = {}
                # AV rounds lag the score/exp stream by AV_LAG slots so the
                # next head's scores issue before this head's tail AV matmuls
                # (keeps the scalar engine's exp stream gapless); bgq drips
                # norm/out-proj PE work one small closure per round slot.
                pending = deque()
                bgq = deque()
                AV_LAG = 2

                def drain_slot():
                    if len(pending) > AV_LAG:
                        pending.popleft()()
                    if bgq:
                        bgq.popleft()()

                def attn_head(qb, h, mts, rsall, uovs):
                    ctx = {}
                    for r in range(NR):
                        ss = psS.tile([128, RKT, QW], F32, tag="psS",
                                      name=f"ss{qb}_{h}_{r}")
                        for jj in range(RKT):
                            j = r * RKT + jj
                            nc.tensor.matmul(
                                ss[:, jj, :],
                                qkT[0:DH, 1, h, j * 128:(j + 1) * 128],
                                qkT[0:DH, 0, h, qb * QW:(qb + 1) * QW],
                                start=True, stop=True)
                        pt = pp.tile([128, RKT, QW], BF16, tag="pT",
                                     name=f"pt{qb}_{h}_{r}")
                        nc.scalar.activation(
                            pt[:], ss[:],
                            mybir.ActivationFunctionType.Exp,
                            scale=inv_sqrt)
                        mt = mts[r // (NR // 2)]
                        rr = r % (NR // 2)
                        nc.vector.tensor_mul(
                            pt[:], pt[:], mt[:, rr * RKT:(rr + 1) * RKT, :])

                        def av(r=r, pt=pt, qb=qb, h=h, ctx=ctx):
                            if "ov" not in ctx:
                                ctx["ov"] = psA.tile(
                                    [128, QW], F32, tag="psA",
                                    name=f"ov{qb}_{h}")
                            ov = ctx["ov"]
                            for jj in range(RKT):
                                j = r * RKT + jj
                                nc.tensor.matmul(
                                    ov[0:DH + 1, :], vh[j][:, h, :],
                                    pt[:, jj, :],
                                    start=(j == 0), stop=(j == KT - 1))
                        pending.append(av)
                        drain_slot()

                    def evac(qb=qb, h=h, ctx=ctx, rsall=rsall, uovs=uovs):
                        ov = ctx["ov"]
                        uov = pt2.tile([DH, QW], BF16, tag="uov",
                                       name=f"uov{qb}_{h}", bufs=2 * HPC + 1)
                        nc.vector.tensor_copy(uov[:], ov[0:DH, :])
                        rs2 = pt2.tile([66, QW], F32, tag="rs2",
                                       name=f"rs2_{qb}_{h}", bufs=2)
                        nc.vector.tensor_copy(rs2[64:66, :], ov[64:66, :])
                        nc.gpsimd.dma_start(rsall[h:h + 1, :], rs2[65:66, :])
                        uovs.append(uov)
                    pending.append(evac)
                    # heads push 9 closures over 8 round slots; one extra pop
                    # here keeps the queue depth (and the AV lag) constant
                    if len(pending) > AV_LAG:
                        pending.popleft()()

                def norm_head_cl(qb, h):
                    def cl():
                        st = state[qb]
                        if "rcall" not in st:
                            rcall = pt2.tile([HPC, QW], F32, tag="rcall",
                                             name=f"rcall{qb}", bufs=2)
                            nc.vector.reciprocal(rcall[:], st["rsall"][:])
                            # bf16 copy so the broadcast matmul below is not
                            # a pipeline-poisoning fp32 matmul
                            rcb = pt2.tile([HPC, QW], BF16, tag="rcb",
                                           name=f"rcb{qb}", bufs=2)
                            nc.vector.tensor_copy(rcb[:], rcall[:])
                            st["rcall"] = rcb
                        rbp = psA.tile([128, QW], F32, tag="psA",
                                       name=f"rbp{qb}_{h}")
                        nc.tensor.matmul(rbp[0:DH, :],
                                         sel8[:, h * DH:(h + 1) * DH],
                                         st["rcall"][:],
                                         start=True, stop=True)
                        cch = pt2.tile([DH, QW], BF16, tag="cch",
                                       name=f"cch{qb}_{h}", bufs=2)
                        nc.vector.tensor_mul(cch[:], rbp[0:DH, :],
                                             st["uovs"][h][:])
                        # pack into 128-row concatT tiles (DMA shifts rows)
                        r0 = h * DH
                        for i, (a, b) in enumerate(PKT):
                            lo, hi = max(r0, a), min(r0 + DH, b)
                            if lo < hi:
                                nc.gpsimd.dma_start(
                                    ccp[i][lo - a:hi - a,
                                           qb * QW:(qb + 1) * QW],
                                    cch[lo - r0:hi - r0, :])
                    return cl

                def outproj_cls(m):
                    ctx = {}

                    def mk_mm(n, i, a, b):
                        def cl():
                            if n not in ctx:
                                ctx[n] = psA.tile([128, QW], F32, tag="psA",
                                                  name=f"psop{m}_{n}")
                            nc.tensor.matmul(
                                ctx[n][:], ccp[i][:, m * 128:(m + 1) * 128],
                                wop[i][:, n * QW:(n + 1) * QW],
                                start=(i == 0), stop=(i == len(PKT) - 1))
                        return cl

                    def mk_copy(n):
                        def cl():
                            if "osb" not in ctx:
                                ctx["osb"] = po.tile([128, D], F32, tag="osb",
                                                     name=f"osb{m}")
                            nc.vector.tensor_copy(
                                ctx["osb"][:, n * QW:(n + 1) * QW],
                                ctx[n][:])
                        return cl

                    def mk_dma():
                        def cl():
                            nc.sync.dma_start(
                                out[m * 128:(m + 1) * 128, :], ctx["osb"][:])
                        return cl

                    cls = []
                    for n in range(2):
                        for i, (a, b) in enumerate(PKT):
                            cls.append(mk_mm(n, i, a, b))
                        cls.append(mk_copy(n))
                    cls.append(mk_dma())
                    return cls

                for qb in range(QB):
                    mts = []
                    for hf in range(2):
                        mt = pm.tile([128, KT // 2, QW], BF16, tag="mask",
                                     name=f"mask{qb}_{hf}")
                        nc.sync.dma_start(
                            mt[:], mh[qb, :, hf * (KT // 2) * QW:
                                      (hf + 1) * (KT // 2) * QW])
                        mts.append(mt)
                    rsall = pt2.tile([HPC, QW], F32, tag="rsall",
                                     name=f"rsall{qb}", bufs=2)
                    uovs = []
                    state[qb] = {"rsall": rsall, "uovs": uovs}
                    for h in range(HPC):
                        attn_head(qb, h, mts, rsall, uovs)
                        if qb > 0 and h == 0:
                            # safe now: evac(qb-1, h7) popped during h0,
                            # so rsall/uovs of qb-1 are fully emitted
                            for hh in range(HPC):
                                bgq.append(norm_head_cl(qb - 1, hh))
                            for m in range((qb - 1) * 4, qb * 4):
                                bgq.extend(outproj_cls(m))
                while pending:
                    pending.popleft()()
                for h in range(HPC):
                    bgq.append(norm_head_cl(QB - 1, h))
                for m in range((QB - 1) * 4, QB * 4):
                    bgq.extend(outproj_cls(m))
                while bgq:
                    bgq.popleft()()

    return nc


def _prep_inputs(q, k, v, mask, Wq, bqv, Wk, bkv, Wv, bvv, Wo):
    """Per-core input maps (numpy, host-side shard + cast)."""
    in_maps = []
    sel8 = np.zeros((HPC, HPC * DH), np.float32)
    for h in range(HPC):
        sel8[h, h * DH:(h + 1) * DH] = 1.0
    sel8 = sel8.astype(BF)
    mask_h = {}
    for b in range(B):
        mt = (mask[b, 0] != 0).astype(np.float32).T  # [k, q]
        m4 = mt.reshape(KT, 128, QB, QW).transpose(2, 1, 0, 3)
        mask_h[b] = np.ascontiguousarray(m4.reshape(QB, 128, KT * QW)).astype(BF)
    for c in range(N_CORES):
        b, hh = c // 2, c % 2
        sl = slice(hh * PC, (hh + 1) * PC)
        in_maps.append({
            "xq": np.ascontiguousarray(q[b].T).astype(BF),
            "xk": np.ascontiguousarray(k[b].T).astype(BF),
            "xv": np.ascontiguousarray(v[b].T).astype(BF),
            "maskH": mask_h[b],
            "wqT": np.ascontiguousarray(Wq[sl, :].T).astype(BF),
            "wkT": np.ascontiguousarray(Wk[sl, :].T).astype(BF),
            "wvT": np.ascontiguousarray(Wv[sl, :].T).astype(BF),
            "bqT": np.ascontiguousarray(
                bqv[sl].reshape(HPC, DH).T).astype(np.float32),
            "bkT": np.ascontiguousarray(
                bkv[sl].reshape(HPC, DH).T).astype(np.float32),
            "bv": bvv[sl].reshape(1, PC).astype(BF),
            "woT": np.ascontiguousarray(Wo[:, sl].T).astype(BF),
            "sel8": sel8,
        })
    return in_maps


def run_sharded(in_maps, **kwargs):
    if "nc" not in _BUILT:
        _BUILT["nc"] = _build_nc()
    return run_bass_kernel_spmd(_BUILT["nc"], in_maps,
                                core_ids=list(range(N_CORES)), **kwargs)


def kernel(q, k, v, mask, Wq, bq, Wk, bk, Wv, bv, Wo, bo):
    q = np.asarray(q, np.float32)
    k = np.asarray(k, np.float32)
    v = np.asarray(v, np.float32)
    mask = np.asarray(mask)
    in_maps = _prep_inputs(q, k, v, mask,
                           np.asarray(Wq, np.float32), np.asarray(bq, np.float32),
                           np.asarray(Wk, np.float32), np.asarray(bk, np.float32),
                           np.asarray(Wv, np.float32), np.asarray(bv, np.float32),
                           np.asarray(Wo, np.float32))
    res = run_sharded(in_maps)
    bo32 = np.asarray(bo, np.float32)
    out = np.empty((B, S, D), np.float32)
    for b in range(B):
        out[b] = res.results[2 * b]["out"] + res.results[2 * b + 1]["out"] + bo32
    return out


# revision 29
# speedup vs baseline: 1.1507x; 1.0877x over previous
"""Multi-head attention (B=4,S=2048,D=1024,H=16,dh=65) on 8 TRN2 NeuronCores.

Sharding: batch x head-half. Core c handles batch c//2 and heads
(c%2)*8..(c%2)*8+8 (P-slice of 520). Each core computes its QKV projections,
attention, and a partial out-projection; the host sums the two partials per
batch and adds bo.

v2 layout: Q/K projections are computed directly in transposed [dh, s] form
(per-head weight slice stationary, x moving with N=2048 streams) which
removes all PE transposes and PSUM-shuffle copies of v1; V stays s-major
(x stationary) for the attention AV matmul. Softmax runs unnormalized in
bf16 with the row-sum harvested from a trailing ones-column in V.
"""

import math
import sys
from collections import deque
from contextlib import ExitStack

import numpy as np
import ml_dtypes

sys.path.insert(0, "/opt/trn_rl_repo")

import concourse.bass as bass
import concourse.mybir as mybir
import concourse.tile as tile_mod
from concourse.bass_utils import run_bass_kernel_spmd
from concourse.vector_clock import ScopedClock

# ---------------------------------------------------------------------------
# Patch for this container's walrus build: it rejects instructions carrying
# more than one semaphore wait ("Too many sync wait commands"), but Tile's
# wait assigner freely attaches several. Split excess waits onto bass_nofuse
# InstNoOp carriers on the same engine, committed immediately before the
# instruction (same-engine program order => over-synchronization only).
# ---------------------------------------------------------------------------
_MAX_WAITS = 1

_orig_commit = tile_mod.TileContext._commit_instruction


def _split_waits(self, inst, commit):
    si = inst.sync_info
    if si is None or len(si.on_wait) <= _MAX_WAITS:
        return
    waits = list(si.on_wait)
    sem_w = [w for w in waits if getattr(w, "sync_type", "semaphore") == "semaphore"]
    other_w = [w for w in waits if getattr(w, "sync_type", "semaphore") != "semaphore"]
    keep_budget = _MAX_WAITS - len(other_w)
    if keep_budget < 0:
        return
    keep = other_w + (sem_w[-keep_budget:] if keep_budget > 0 else [])
    excess = sem_w[: len(sem_w) - max(keep_budget, 0)]
    if not excess:
        return
    for i, w in enumerate(excess):
        nop = mybir.InstNoOp(
            name=f"{inst.name}-sw{i}",
            sync_info=mybir.SyncInfo(on_wait=[w], on_update=[]),
            bass_nofuse=True,
            engine=inst.engine,
        )
        commit(nop)
    inst.sync_info = mybir.SyncInfo(on_wait=keep, on_update=list(si.on_update))


def _patched_commit(self, inst, lazy_reg_writes: bool = True):
    if inst.engine != mybir.EngineType.Unassigned:
        _split_waits(self, inst, lambda n: _orig_commit(self, n, False))
    return _orig_commit(self, inst, lazy_reg_writes)


def _patched_drain_and_barrier(self, tick_clock, wait_clock):
    drain_inst = self.nc.sync.drain()
    wait_clock.add_sem_waits(
        drain_inst.ins, ScopedClock({None: tick_clock.global_clock})
    )
    si = drain_inst.ins.sync_info
    if si is not None and len(si.on_wait) > _MAX_WAITS:
        waits = list(si.on_wait)
        drain_inst.ins.sync_info = mybir.SyncInfo(
            on_wait=waits[:_MAX_WAITS], on_update=list(si.on_update)
        )
        for w in waits[_MAX_WAITS:]:
            n = self.nc.sync.nop(nofuse=True)
            n.ins.sync_info = mybir.SyncInfo(on_wait=[w], on_update=[])
    self.nc.all_engine_barrier()
    popped = self.nc._tile_sem_poison_stack.pop()
    assert popped is self._sem_poison
    self.nc.clear_and_free_semaphores(list(self.sems.allocated().values()))
    self.nc.all_engine_barrier()


tile_mod.TileContext._commit_instruction = _patched_commit
tile_mod.TileContext._drain_and_barrier = _patched_drain_and_barrier

# ---------------------------------------------------------------------------

B, S, D, H = 4, 2048, 1024, 16
DH = D // H + 1          # 65
P = H * DH               # 1040
HPC = H // 2             # heads per core
PC = HPC * DH            # 520, per-core P slice
N_CORES = 8

MT = S // 128            # 16 row blocks / k tiles
KT = 16                  # k tiles per attention
QB = 4                   # q blocks of 512
QW = 512
RKT = 2                  # k-tiles per score round (2 banks, double-buffered)
NR = KT // RKT           # 8 rounds

F32 = mybir.dt.float32
BF16 = mybir.dt.bfloat16
BF = ml_dtypes.bfloat16

_BUILT = {}


def _build_nc():
    nc = bass.Bass("TRN2", target_bir_lowering=False, debug=False,
                   num_devices=N_CORES)

    xq_d = nc.dram_tensor("xq", [D, S], BF16, kind="ExternalInput").ap()
    xk_d = nc.dram_tensor("xk", [D, S], BF16, kind="ExternalInput").ap()
    xv_d = nc.dram_tensor("xv", [D, S], BF16, kind="ExternalInput").ap()
    # maskH[qb, p, j*QW+q] = maskT[j*128+p, qb*512+q] (multiplicative 0/1)
    mh = nc.dram_tensor("maskH", [QB, 128, KT * QW], BF16,
                        kind="ExternalInput").ap()
    # head-pair packed K/Q weights: pair p holds heads 2p|2p+1, dh 0..63
    # each; the dh64 rows of all 8 heads form the 8-col straggler tensor
    wqP_d = nc.dram_tensor("wqP", [D, 4, 128], BF16, kind="ExternalInput").ap()
    wkP_d = nc.dram_tensor("wkP", [D, 4, 128], BF16, kind="ExternalInput").ap()
    wq64_d = nc.dram_tensor("wq64", [D, HPC], BF16, kind="ExternalInput").ap()
    wk64_d = nc.dram_tensor("wk64", [D, HPC], BF16, kind="ExternalInput").ap()
    bqP_d = nc.dram_tensor("bqP", [128, 4], F32, kind="ExternalInput").ap()
    bkP_d = nc.dram_tensor("bkP", [128, 4], F32, kind="ExternalInput").ap()
    bq64_d = nc.dram_tensor("bq64", [HPC, 1], F32, kind="ExternalInput").ap()
    bk64_d = nc.dram_tensor("bk64", [HPC, 1], F32, kind="ExternalInput").ap()
    # V weights packed 8 heads x dh0..63 + dh64 straggler
    wvP_d = nc.dram_tensor("wvP", [D, 512], BF16, kind="ExternalInput").ap()
    wv64_d = nc.dram_tensor("wv64", [D, HPC], BF16, kind="ExternalInput").ap()
    bvP_d = nc.dram_tensor("bvP", [1, 512], BF16, kind="ExternalInput").ap()
    bv64_d = nc.dram_tensor("bv64", [1, HPC], BF16, kind="ExternalInput").ap()
    wo_d = nc.dram_tensor("woT", [PC, D], BF16, kind="ExternalInput").ap()
    sel8_d = nc.dram_tensor("sel8", [HPC, HPC * DH], BF16,
                            kind="ExternalInput").ap()
    out = nc.dram_tensor("out", [S, D], F32, kind="ExternalOutput").ap()

    # packed 128-row k-tile ranges of the 520-row concatT / WoT
    PKT = [(0, 128), (128, 256), (256, 384), (384, 512), (512, 520)]

    with tile_mod.TileContext(nc) as tc:
        with tc.tile_pool(name="const", bufs=1) as pconst, \
             tc.tile_pool(name="qkT", bufs=1) as pqkT, \
             tc.tile_pool(name="vh", bufs=MT + 1) as pvh:

            ones_col = pconst.tile([1, 128], BF16, tag="ones")
            nc.gpsimd.memset(ones_col[:], 1.0)
            sel8h = []
            for half in range(2):
                t = pconst.tile([4, HPC * DH], BF16, tag=f"sel8{half}",
                                name=f"sel8{half}")
                nc.sync.dma_start(t[:], sel8_d[half * 4:half * 4 + 4, :])
                sel8h.append(t)

            # [65, proj(q=0,k=1), head, S]
            qkT = pqkT.tile([DH, 2, HPC, S], BF16, tag="qkT")
            # v k-tiles with trailing ones column: [128, head, 65+1]
            vh = [pvh.tile([128, HPC, DH + 1], BF16, tag="vh", name=f"vh{j}")
                  for j in range(MT)]
            for j in range(MT):
                nc.gpsimd.memset(vh[j][:, :, DH:DH + 1], 1.0)

            # --------------- phase 1a: V projection (s-major) --------------
            _xw_stack = ExitStack()
            px = _xw_stack.enter_context(tc.tile_pool(name="px", bufs=16))
            pw = _xw_stack.enter_context(tc.tile_pool(name="pw", bufs=16))
            pb = _xw_stack.enter_context(tc.tile_pool(name="pb", bufs=1))

            def load_x(xd, tag):
                # two half-tile DMAs per d so the first row-blocks land early
                xts = [px.tile([128, S], BF16, tag="x", name=f"x{tag}{d}")
                       for d in range(8)]
                for c in range(2):
                    for d in range(8):
                        nc.sync.dma_start(
                            xts[d][:, c * 1024:(c + 1) * 1024],
                            xd[d * 128:(d + 1) * 128,
                               c * 1024:(c + 1) * 1024])
                return xts

            def load_w(wd, shape, tag):
                # weights ride the gpsimd DMA queue so they are not stuck
                # behind the big x transfers on the sync queue
                wts = []
                for d in range(8):
                    wt = pw.tile([128] + shape, BF16, tag=f"w{tag}",
                                 name=f"w{tag}{d}")
                    nc.gpsimd.dma_start(wt[:], wd[d * 128:(d + 1) * 128])
                    wts.append(wt)
                return wts

            wvP_t = load_w(wvP_d, [512], "vP")
            wv64_t = load_w(wv64_d, [HPC], "v64")
            bvP_t = pb.tile([1, 512], BF16, tag="bvP")
            nc.gpsimd.dma_start(bvP_t[:], bvP_d[:])
            bv64_t = pb.tile([1, HPC], BF16, tag="bv64")
            nc.gpsimd.dma_start(bv64_t[:], bv64_d[:])
            wkP_t = load_w(wkP_d, [4, 128], "kP")
            wk64_t = load_w(wk64_d, [HPC], "k64")
            wqP_t = load_w(wqP_d, [4, 128], "qP")
            wq64_t = load_w(wq64_d, [HPC], "q64")
            bkP_t = pb.tile([128, 4], F32, tag="bkP")
            nc.gpsimd.dma_start(bkP_t[:], bkP_d[:])
            bqP_t = pb.tile([128, 4], F32, tag="bqP")
            nc.gpsimd.dma_start(bqP_t[:], bqP_d[:])
            bk64_t = pb.tile([HPC, 1], F32, tag="bk64")
            nc.gpsimd.dma_start(bk64_t[:], bk64_d[:])
            bq64_t = pb.tile([HPC, 1], F32, tag="bq64")
            nc.gpsimd.dma_start(bq64_t[:], bq64_d[:])
            xv_t = load_x(xv_d, "v")
            # K/Q inputs: start their DMAs now so they land during V math
            xk_t = load_x(xk_d, "k")

            with tc.tile_pool(name="psV", bufs=2, space="PSUM") as psV:
                for m in range(MT):
                    psa = psV.tile([128, HPC, 64], F32, tag="psVa",
                                   name=f"psva{m}")
                    psb = psV.tile([128, HPC, 1], F32, tag="psVb",
                                   name=f"psvb{m}")
                    nc.tensor.matmul(psa[:], ones_col[0:1, :],
                                     bvP_t[0:1, :], start=True, stop=False)
                    nc.tensor.matmul(psb[:], ones_col[0:1, :],
                                     bv64_t[0:1, :], start=True, stop=False)
                    for d in range(8):
                        nc.tensor.matmul(
                            psa[:], xv_t[d][:, m * 128:(m + 1) * 128],
                            wvP_t[d][:], start=False, stop=(d == 7))
                        nc.tensor.matmul(
                            psb[:], xv_t[d][:, m * 128:(m + 1) * 128],
                            wv64_t[d][:], start=False, stop=(d == 7))
                    nc.vector.tensor_copy(vh[m][:, :, 0:64], psa[:])
                    nc.vector.tensor_copy(vh[m][:, :, 64:65], psb[:])

            # xq reuses the V buffers (auto-synced on last V-proj reader)
            xq_t = load_x(xq_d, "q")

            # ------- phase 1b: K and Q projections, direct [dh, s] ---------
            # head-pair packing: stationary = [head 2p dh0-63 | head 2p+1
            # dh0-63] (128 cols), plus one 8-col straggler for dh64 of all
            # heads; evacuation DMA-shifts the upper half down to partition 0
            pevac = _xw_stack.enter_context(tc.tile_pool(name="pevac",
                                                         bufs=1))
            with tc.tile_pool(name="psKQ", bufs=2, space="PSUM") as psKQ:
                for pi, wP, w64, xt, bP, b64 in (
                        (1, wkP_t, wk64_t, xk_t, bkP_t, bk64_t),
                        (0, wqP_t, wq64_t, xq_t, bqP_t, bq64_t)):
                    ps8 = psKQ.tile([HPC, 4, QW], F32, tag="psKQ",
                                    name=f"ps64_{pi}")
                    for d in range(8):
                        for c in range(4):
                            nc.tensor.matmul(
                                ps8[:, c, :], w64[d][:],
                                xt[d][:, c * QW:(c + 1) * QW],
                                start=(d == 0), stop=(d == 7))
                    for c in range(4):
                        tmp8 = pevac.tile([HPC, QW], BF16, tag="tmp8",
                                          bufs=2, name=f"tmp8_{pi}_{c}")
                        nc.vector.tensor_scalar_add(
                            tmp8[:], ps8[:, c, :], b64[:, 0:1])
                        nc.gpsimd.dma_start(
                            qkT[64:65, pi, :, c * QW:(c + 1) * QW],
                            tmp8[:])
                    for p in range(4):
                        ps = psKQ.tile([128, 4, QW], F32, tag="psKQ",
                                       name=f"pskq{pi}_{p}")
                        for d in range(8):
                            for c in range(4):
                                nc.tensor.matmul(
                                    ps[:, c, :], wP[d][:, p, :],
                                    xt[d][:, c * QW:(c + 1) * QW],
                                    start=(d == 0), stop=(d == 7))
                        for c in range(4):
                            tmp = pevac.tile([128, QW], BF16, tag="tmpP",
                                             bufs=4, name=f"tmpP{pi}_{p}_{c}")
                            nc.vector.tensor_scalar_add(
                                tmp[:], ps[:, c, :], bP[:, p:p + 1])
                            nc.gpsimd.dma_start(
                                qkT[0:64, pi, 2 * p, c * QW:(c + 1) * QW],
                                tmp[0:64, :])
                            nc.gpsimd.dma_start(
                                qkT[0:64, pi, 2 * p + 1,
                                    c * QW:(c + 1) * QW],
                                tmp[64:128, :])
            _xw_stack.close()   # free x/w pools before attention phase

            # ---------------- phase 2+3: attention + out-proj --------------
            with tc.tile_pool(name="pm", bufs=3) as pm, \
                 tc.tile_pool(name="pp", bufs=4) as pp, \
                 tc.tile_pool(name="pc", bufs=1) as pc, \
                 tc.tile_pool(name="pwo", bufs=1) as pwo, \
                 tc.tile_pool(name="po", bufs=2) as po, \
                 tc.tile_pool(name="pt2", bufs=4) as pt2, \
                 tc.tile_pool(name="psS", bufs=2, space="PSUM") as psS, \
                 tc.tile_pool(name="psA", bufs=4, space="PSUM") as psA:

                # packed concatT: 128-row tiles covering rows 0..520
                ccp = [pc.tile([b - a, S], BF16, tag=f"ccp{i}",
                               name=f"ccp{i}")
                       for i, (a, b) in enumerate(PKT)]
                wop = []
                for i, (a, b) in enumerate(PKT):
                    w = pwo.tile([b - a, D], BF16, tag=f"wop{i}",
                                 name=f"wop{i}")
                    nc.sync.dma_start(w[:], wo_d[a:b, :])
                    wop.append(w)

                inv_sqrt = 1.0 / math.sqrt(float(DH))
                state = {}
                # AV rounds lag the score/exp stream by AV_LAG slots so the
                # next head's scores issue before this head's tail AV matmuls
                # (keeps the scalar engine's exp stream gapless); bgq drips
                # norm/out-proj PE work one small closure per round slot.
                pending = deque()
                bgq = deque()
                AV_LAG = 2

                def drain_slot():
                    if len(pending) > AV_LAG:
                        pending.popleft()()
                    if bgq:
                        bgq.popleft()()

                def attn_head(qb, h, mts, rsall, uovs):
                    ctx = {}
                    for r in range(NR):
                        ss = psS.tile([128, RKT, QW], F32, tag="psS",
                                      name=f"ss{qb}_{h}_{r}")
                        for jj in range(RKT):
                            j = r * RKT + jj
                            nc.tensor.matmul(
                                ss[:, jj, :],
                                qkT[0:DH, 1, h, j * 128:(j + 1) * 128],
                                qkT[0:DH, 0, h, qb * QW:(qb + 1) * QW],
                                start=True, stop=True)
                        pt = pp.tile([128, RKT, QW], BF16, tag="pT",
                                     name=f"pt{qb}_{h}_{r}")
                        nc.scalar.activation(
                            pt[:], ss[:],
                            mybir.ActivationFunctionType.Exp,
                            scale=inv_sqrt)
                        mt = mts[r // (NR // 2)]
                        rr = r % (NR // 2)
                        nc.vector.tensor_mul(
                            pt[:], pt[:], mt[:, rr * RKT:(rr + 1) * RKT, :])

                        def av(r=r, pt=pt, qb=qb, h=h, ctx=ctx):
                            if "ov" not in ctx:
                                ctx["ov"] = psA.tile(
                                    [128, QW], F32, tag="psA",
                                    name=f"ov{qb}_{h}")
                            ov = ctx["ov"]
                            for jj in range(RKT):
                                j = r * RKT + jj
                                nc.tensor.matmul(
                                    ov[0:DH + 1, :], vh[j][:, h, :],
                                    pt[:, jj, :],
                                    start=(j == 0), stop=(j == KT - 1))
                        pending.append(av)
                        drain_slot()

                    def evac(qb=qb, h=h, ctx=ctx, rsall=rsall, uovs=uovs):
                        ov = ctx["ov"]
                        uov = pt2.tile([DH, QW], BF16, tag="uov",
                                       name=f"uov{qb}_{h}", bufs=2 * HPC + 1)
                        nc.vector.tensor_copy(uov[:], ov[0:DH, :])
                        rs2 = pt2.tile([66, QW], F32, tag="rs2",
                                       name=f"rs2_{qb}_{h}", bufs=2)
                        nc.vector.tensor_copy(rs2[64:66, :], ov[64:66, :])
                        nc.gpsimd.dma_start(
                            rsall[h // 4][h % 4:h % 4 + 1, :],
                            rs2[65:66, :])
                        uovs.append(uov)
                    pending.append(evac)
                    # heads push 9 closures over 8 round slots; one extra pop
                    # here keeps the queue depth (and the AV lag) constant
                    if len(pending) > AV_LAG:
                        pending.popleft()()

                def norm_head_cl(qb, h):
                    def cl():
                        st = state[qb]
                        half = h // 4
                        key = f"rc{half}"
                        if key not in st:
                            rc = pt2.tile([4, QW], F32, tag="rcall",
                                          name=f"rcall{qb}_{half}", bufs=2)
                            nc.vector.reciprocal(
                                rc[:], st["rsall"][half][:])
                            # bf16 copy so the broadcast matmul below is not
                            # a pipeline-poisoning fp32 matmul
                            rcb = pt2.tile([4, QW], BF16, tag="rcb",
                                           name=f"rcb{qb}_{half}", bufs=2)
                            nc.vector.tensor_copy(rcb[:], rc[:])
                            st[key] = rcb
                        rbp = psA.tile([128, QW], F32, tag="psA",
                                       name=f"rbp{qb}_{h}")
                        nc.tensor.matmul(rbp[0:DH, :],
                                         sel8h[half][:, h * DH:(h + 1) * DH],
                                         st[key][:],
                                         start=True, stop=True)
                        cch = pt2.tile([DH, QW], BF16, tag="cch",
                                       name=f"cch{qb}_{h}", bufs=2)
                        nc.vector.tensor_mul(cch[:], rbp[0:DH, :],
                                             st["uovs"][h][:])
                        # pack into 128-row concatT tiles (DMA shifts rows)
                        r0 = h * DH
                        for i, (a, b) in enumerate(PKT):
                            lo, hi = max(r0, a), min(r0 + DH, b)
                            if lo < hi:
                                nc.gpsimd.dma_start(
                                    ccp[i][lo - a:hi - a,
                                           qb * QW:(qb + 1) * QW],
                                    cch[lo - r0:hi - r0, :])
                    return cl

                def outproj_cls(m):
                    ctx = {}

                    def mk_mm(n, i, a, b):
                        def cl():
                            if n not in ctx:
                                ctx[n] = psA.tile([128, QW], F32, tag="psA",
                                                  name=f"psop{m}_{n}")
                            nc.tensor.matmul(
                                ctx[n][:], ccp[i][:, m * 128:(m + 1) * 128],
                                wop[i][:, n * QW:(n + 1) * QW],
                                start=(i == 0), stop=(i == len(PKT) - 1))
                        return cl

                    def mk_copy(n):
                        def cl():
                            if "osb" not in ctx:
                                ctx["osb"] = po.tile([128, D], F32, tag="osb",
                                                     name=f"osb{m}")
                            nc.vector.tensor_copy(
                                ctx["osb"][:, n * QW:(n + 1) * QW],
                                ctx[n][:])
                        return cl

                    def mk_dma():
                        def cl():
                            nc.sync.dma_start(
                                out[m * 128:(m + 1) * 128, :], ctx["osb"][:])
                        return cl

                    cls = []
                    for n in range(2):
                        for i, (a, b) in enumerate(PKT):
                            cls.append(mk_mm(n, i, a, b))
                        cls.append(mk_copy(n))
                    cls.append(mk_dma())
                    return cls

                for qb in range(QB):
                    mts = []
                    for hf in range(2):
                        mt = pm.tile([128, KT // 2, QW], BF16, tag="mask",
                                     name=f"mask{qb}_{hf}")
                        nc.sync.dma_start(
                            mt[:], mh[qb, :, hf * (KT // 2) * QW:
                                      (hf + 1) * (KT // 2) * QW])
                        mts.append(mt)
                    rsall = [pt2.tile([4, QW], F32, tag=f"rsall{half}",
                                      name=f"rsall{qb}_{half}", bufs=2)
                             for half in range(2)]
                    uovs = []
                    state[qb] = {"rsall": rsall, "uovs": uovs}
                    for h in range(HPC):
                        attn_head(qb, h, mts, rsall, uovs)
                        if h == 4:
                            # evac(qb, h3) popped during h4 r1: heads 0-3 of
                            # this qb can normalize while it is still running
                            for hh in range(4):
                                bgq.append(norm_head_cl(qb, hh))
                        if qb > 0 and h == 0:
                            # evac(qb-1, h7) popped during h0 r1
                            for hh in range(4, HPC):
                                bgq.append(norm_head_cl(qb - 1, hh))
                            for m in range((qb - 1) * 4, qb * 4):
                                bgq.extend(outproj_cls(m))
                while pending:
                    pending.popleft()()
                for h in range(4, HPC):
                    bgq.append(norm_head_cl(QB - 1, h))
                for m in range((QB - 1) * 4, QB * 4):
                    bgq.extend(outproj_cls(m))
                while bgq:
                    bgq.popleft()()

    return nc


def _prep_inputs(q, k, v, mask, Wq, bqv, Wk, bkv, Wv, bvv, Wo):
    """Per-core input maps (numpy, host-side shard + cast)."""
    in_maps = []
    sel8 = np.zeros((HPC, HPC * DH), np.float32)
    for h in range(HPC):
        sel8[h, h * DH:(h + 1) * DH] = 1.0
    sel8 = sel8.astype(BF)
    mask_h = {}
    for b in range(B):
        mt = (mask[b, 0] != 0).astype(np.float32).T  # [k, q]
        m4 = mt.reshape(KT, 128, QB, QW).transpose(2, 1, 0, 3)
        mask_h[b] = np.ascontiguousarray(m4.reshape(QB, 128, KT * QW)).astype(BF)
    def pack_pair(Wt, bv_):
        # Wt [D, PC] -> pair-packed [D, 4, 128] + dh64 straggler [D, 8];
        # bias -> [128, 4] pair layout + [8, 1] straggler
        r = Wt.reshape(D, HPC, DH)
        # wP[:, p] = [head2p dh0-63 | head2p+1 dh0-63]
        wP = np.stack([np.concatenate([r[:, 2 * p, :64],
                                       r[:, 2 * p + 1, :64]], axis=1)
                       for p in range(4)], axis=1)
        w64 = r[:, :, 64]
        br = bv_.reshape(HPC, DH)
        bP = np.stack([np.concatenate([br[2 * p, :64], br[2 * p + 1, :64]])
                       for p in range(4)], axis=1)
        b64 = br[:, 64:65]
        return (np.ascontiguousarray(wP).astype(BF),
                np.ascontiguousarray(w64).astype(BF),
                np.ascontiguousarray(bP).astype(np.float32),
                np.ascontiguousarray(b64).astype(np.float32))

    for c in range(N_CORES):
        b, hh = c // 2, c % 2
        sl = slice(hh * PC, (hh + 1) * PC)
        wqP, wq64, bqP, bq64 = pack_pair(
            np.ascontiguousarray(Wq[sl, :].T), bqv[sl])
        wkP, wk64, bkP, bk64 = pack_pair(
            np.ascontiguousarray(Wk[sl, :].T), bkv[sl])
        rv = np.ascontiguousarray(Wv[sl, :].T).reshape(D, HPC, DH)
        bvr = bvv[sl].reshape(HPC, DH)
        in_maps.append({
            "xq": np.ascontiguousarray(q[b].T).astype(BF),
            "xk": np.ascontiguousarray(k[b].T).astype(BF),
            "xv": np.ascontiguousarray(v[b].T).astype(BF),
            "maskH": mask_h[b],
            "wqP": wqP, "wq64": wq64, "bqP": bqP, "bq64": bq64,
            "wkP": wkP, "wk64": wk64, "bkP": bkP, "bk64": bk64,
            "wvP": np.ascontiguousarray(
                rv[:, :, :64].reshape(D, 512)).astype(BF),
            "wv64": np.ascontiguousarray(rv[:, :, 64]).astype(BF),
            "bvP": bvr[:, :64].reshape(1, 512).astype(BF),
            "bv64": bvr[:, 64].reshape(1, HPC).astype(BF),
            "woT": np.ascontiguousarray(Wo[:, sl].T).astype(BF),
            "sel8": sel8,
        })
    return in_maps


def run_sharded(in_maps, **kwargs):
    if "nc" not in _BUILT:
        _BUILT["nc"] = _build_nc()
    return run_bass_kernel_spmd(_BUILT["nc"], in_maps,
                                core_ids=list(range(N_CORES)), **kwargs)


def kernel(q, k, v, mask, Wq, bq, Wk, bk, Wv, bv, Wo, bo):
    q = np.asarray(q, np.float32)
    k = np.asarray(k, np.float32)
    v = np.asarray(v, np.float32)
    mask = np.asarray(mask)
    in_maps = _prep_inputs(q, k, v, mask,
                           np.asarray(Wq, np.float32), np.asarray(bq, np.float32),
                           np.asarray(Wk, np.float32), np.asarray(bk, np.float32),
                           np.asarray(Wv, np.float32), np.asarray(bv, np.float32),
                           np.asarray(Wo, np.float32))
    res = run_sharded(in_maps)
    bo32 = np.asarray(bo, np.float32)
    out = np.empty((B, S, D), np.float32)
    for b in range(B):
        out[b] = res.results[2 * b]["out"] + res.results[2 * b + 1]["out"] + bo32
    return out


# revision 31
# speedup vs baseline: 1.1519x; 1.0011x over previous
"""Multi-head attention (B=4,S=2048,D=1024,H=16,dh=65) on 8 TRN2 NeuronCores.

Sharding: batch x head-half. Core c handles batch c//2 and heads
(c%2)*8..(c%2)*8+8 (P-slice of 520). Each core computes its QKV projections,
attention, and a partial out-projection; the host sums the two partials per
batch and adds bo.

v2 layout: Q/K projections are computed directly in transposed [dh, s] form
(per-head weight slice stationary, x moving with N=2048 streams) which
removes all PE transposes and PSUM-shuffle copies of v1; V stays s-major
(x stationary) for the attention AV matmul. Softmax runs unnormalized in
bf16 with the row-sum harvested from a trailing ones-column in V.
"""

import math
import sys
from collections import deque
from contextlib import ExitStack

import numpy as np
import ml_dtypes

sys.path.insert(0, "/opt/trn_rl_repo")

import concourse.bass as bass
import concourse.mybir as mybir
import concourse.tile as tile_mod
from concourse.bass_utils import run_bass_kernel_spmd
from concourse.vector_clock import ScopedClock

# ---------------------------------------------------------------------------
# Patch for this container's walrus build: it rejects instructions carrying
# more than one semaphore wait ("Too many sync wait commands"), but Tile's
# wait assigner freely attaches several. Split excess waits onto bass_nofuse
# InstNoOp carriers on the same engine, committed immediately before the
# instruction (same-engine program order => over-synchronization only).
# ---------------------------------------------------------------------------
_MAX_WAITS = 1

_orig_commit = tile_mod.TileContext._commit_instruction


def _split_waits(self, inst, commit):
    si = inst.sync_info
    if si is None or len(si.on_wait) <= _MAX_WAITS:
        return
    waits = list(si.on_wait)
    sem_w = [w for w in waits if getattr(w, "sync_type", "semaphore") == "semaphore"]
    other_w = [w for w in waits if getattr(w, "sync_type", "semaphore") != "semaphore"]
    keep_budget = _MAX_WAITS - len(other_w)
    if keep_budget < 0:
        return
    keep = other_w + (sem_w[-keep_budget:] if keep_budget > 0 else [])
    excess = sem_w[: len(sem_w) - max(keep_budget, 0)]
    if not excess:
        return
    for i, w in enumerate(excess):
        nop = mybir.InstNoOp(
            name=f"{inst.name}-sw{i}",
            sync_info=mybir.SyncInfo(on_wait=[w], on_update=[]),
            bass_nofuse=True,
            engine=inst.engine,
        )
        commit(nop)
    inst.sync_info = mybir.SyncInfo(on_wait=keep, on_update=list(si.on_update))


def _patched_commit(self, inst, lazy_reg_writes: bool = True):
    if inst.engine != mybir.EngineType.Unassigned:
        _split_waits(self, inst, lambda n: _orig_commit(self, n, False))
    return _orig_commit(self, inst, lazy_reg_writes)


def _patched_drain_and_barrier(self, tick_clock, wait_clock):
    drain_inst = self.nc.sync.drain()
    wait_clock.add_sem_waits(
        drain_inst.ins, ScopedClock({None: tick_clock.global_clock})
    )
    si = drain_inst.ins.sync_info
    if si is not None and len(si.on_wait) > _MAX_WAITS:
        waits = list(si.on_wait)
        drain_inst.ins.sync_info = mybir.SyncInfo(
            on_wait=waits[:_MAX_WAITS], on_update=list(si.on_update)
        )
        for w in waits[_MAX_WAITS:]:
            n = self.nc.sync.nop(nofuse=True)
            n.ins.sync_info = mybir.SyncInfo(on_wait=[w], on_update=[])
    self.nc.all_engine_barrier()
    popped = self.nc._tile_sem_poison_stack.pop()
    assert popped is self._sem_poison
    self.nc.clear_and_free_semaphores(list(self.sems.allocated().values()))
    self.nc.all_engine_barrier()


tile_mod.TileContext._commit_instruction = _patched_commit
tile_mod.TileContext._drain_and_barrier = _patched_drain_and_barrier

# ---------------------------------------------------------------------------

B, S, D, H = 4, 2048, 1024, 16
DH = D // H + 1          # 65
P = H * DH               # 1040
HPC = H // 2             # heads per core
PC = HPC * DH            # 520, per-core P slice
N_CORES = 8

MT = S // 128            # 16 row blocks / k tiles
KT = 16                  # k tiles per attention
QB = 4                   # q blocks of 512
QW = 512
RKT = 2                  # k-tiles per score round (2 banks, double-buffered)
NR = KT // RKT           # 8 rounds

F32 = mybir.dt.float32
BF16 = mybir.dt.bfloat16
BF = ml_dtypes.bfloat16

_BUILT = {}


def _build_nc():
    nc = bass.Bass("TRN2", target_bir_lowering=False, debug=False,
                   num_devices=N_CORES)

    xq_d = nc.dram_tensor("xq", [D, S], BF16, kind="ExternalInput").ap()
    xk_d = nc.dram_tensor("xk", [D, S], BF16, kind="ExternalInput").ap()
    xv_d = nc.dram_tensor("xv", [D, S], BF16, kind="ExternalInput").ap()
    # maskH[qb, p, j*QW+q] = maskT[j*128+p, qb*512+q] (multiplicative 0/1)
    mh = nc.dram_tensor("maskH", [QB, 128, KT * QW], BF16,
                        kind="ExternalInput").ap()
    # head-pair packed K/Q weights: pair p holds heads 2p|2p+1, dh 0..63
    # each; the dh64 rows of all 8 heads form the 8-col straggler tensor
    wqP_d = nc.dram_tensor("wqP", [D, 4, 128], BF16, kind="ExternalInput").ap()
    wkP_d = nc.dram_tensor("wkP", [D, 4, 128], BF16, kind="ExternalInput").ap()
    wq64_d = nc.dram_tensor("wq64", [D, HPC], BF16, kind="ExternalInput").ap()
    wk64_d = nc.dram_tensor("wk64", [D, HPC], BF16, kind="ExternalInput").ap()
    bqP_d = nc.dram_tensor("bqP", [128, 4], F32, kind="ExternalInput").ap()
    bkP_d = nc.dram_tensor("bkP", [128, 4], F32, kind="ExternalInput").ap()
    bq64_d = nc.dram_tensor("bq64", [HPC, 1], F32, kind="ExternalInput").ap()
    bk64_d = nc.dram_tensor("bk64", [HPC, 1], F32, kind="ExternalInput").ap()
    # V weights packed 8 heads x dh0..63 + dh64 straggler
    wvP_d = nc.dram_tensor("wvP", [D, 512], BF16, kind="ExternalInput").ap()
    wv64_d = nc.dram_tensor("wv64", [D, HPC], BF16, kind="ExternalInput").ap()
    bvP_d = nc.dram_tensor("bvP", [1, 512], BF16, kind="ExternalInput").ap()
    bv64_d = nc.dram_tensor("bv64", [1, HPC], BF16, kind="ExternalInput").ap()
    wo_d = nc.dram_tensor("woT", [PC, D], BF16, kind="ExternalInput").ap()
    sel8_d = nc.dram_tensor("sel8", [HPC, HPC * DH], BF16,
                            kind="ExternalInput").ap()
    out = nc.dram_tensor("out", [S, D], F32, kind="ExternalOutput").ap()

    # packed 128-row k-tile ranges of the 520-row concatT / WoT
    PKT = [(0, 128), (128, 256), (256, 384), (384, 512), (512, 520)]

    with tile_mod.TileContext(nc) as tc:
        with tc.tile_pool(name="const", bufs=1) as pconst, \
             tc.tile_pool(name="qkT", bufs=1) as pqkT, \
             tc.tile_pool(name="vh", bufs=MT + 1) as pvh:

            ones_col = pconst.tile([1, 128], BF16, tag="ones")
            nc.gpsimd.memset(ones_col[:], 1.0)
            sel8h = [pconst.tile([4, HPC * DH], BF16, tag=f"sel8{half}",
                                 name=f"sel8{half}")
                     for half in range(2)]

            # [65, proj(q=0,k=1), head, S]
            qkT = pqkT.tile([DH, 2, HPC, S], BF16, tag="qkT")
            # v k-tiles with trailing ones column: [128, head, 65+1]
            vh = [pvh.tile([128, HPC, DH + 1], BF16, tag="vh", name=f"vh{j}")
                  for j in range(MT)]
            for j in range(MT):
                nc.gpsimd.memset(vh[j][:, :, DH:DH + 1], 1.0)

            # --------------- phase 1a: V projection (s-major) --------------
            _xw_stack = ExitStack()
            px = _xw_stack.enter_context(tc.tile_pool(name="px", bufs=16))
            pw = _xw_stack.enter_context(tc.tile_pool(name="pw", bufs=16))
            pb = _xw_stack.enter_context(tc.tile_pool(name="pb", bufs=1))

            def load_x(xd, tag):
                # two half-tile DMAs per d so the first row-blocks land early
                xts = [px.tile([128, S], BF16, tag="x", name=f"x{tag}{d}")
                       for d in range(8)]
                for c in range(2):
                    for d in range(8):
                        nc.sync.dma_start(
                            xts[d][:, c * 1024:(c + 1) * 1024],
                            xd[d * 128:(d + 1) * 128,
                               c * 1024:(c + 1) * 1024])
                return xts

            def load_w(wd, shape, tag):
                # weights ride the gpsimd DMA queue so they are not stuck
                # behind the big x transfers on the sync queue
                wts = []
                for d in range(8):
                    wt = pw.tile([128] + shape, BF16, tag=f"w{tag}",
                                 name=f"w{tag}{d}")
                    nc.gpsimd.dma_start(wt[:], wd[d * 128:(d + 1) * 128])
                    wts.append(wt)
                return wts

            wvP_t = load_w(wvP_d, [512], "vP")
            wv64_t = load_w(wv64_d, [HPC], "v64")
            bvP_t = pb.tile([1, 512], BF16, tag="bvP")
            nc.gpsimd.dma_start(bvP_t[:], bvP_d[:])
            bv64_t = pb.tile([1, HPC], BF16, tag="bv64")
            nc.gpsimd.dma_start(bv64_t[:], bv64_d[:])
            wkP_t = load_w(wkP_d, [4, 128], "kP")
            wk64_t = load_w(wk64_d, [HPC], "k64")
            wqP_t = load_w(wqP_d, [4, 128], "qP")
            wq64_t = load_w(wq64_d, [HPC], "q64")
            bkP_t = pb.tile([128, 4], F32, tag="bkP")
            nc.gpsimd.dma_start(bkP_t[:], bkP_d[:])
            bqP_t = pb.tile([128, 4], F32, tag="bqP")
            nc.gpsimd.dma_start(bqP_t[:], bqP_d[:])
            bk64_t = pb.tile([HPC, 1], F32, tag="bk64")
            nc.gpsimd.dma_start(bk64_t[:], bk64_d[:])
            bq64_t = pb.tile([HPC, 1], F32, tag="bq64")
            nc.gpsimd.dma_start(bq64_t[:], bq64_d[:])
            for half in range(2):
                nc.gpsimd.dma_start(sel8h[half][:],
                                    sel8_d[half * 4:half * 4 + 4, :])
            xv_t = load_x(xv_d, "v")
            # K/Q inputs: start their DMAs now so they land during V math
            xk_t = load_x(xk_d, "k")

            with tc.tile_pool(name="psV", bufs=2, space="PSUM") as psV:
                for m in range(MT):
                    psa = psV.tile([128, HPC, 64], F32, tag="psVa",
                                   name=f"psva{m}")
                    psb = psV.tile([128, HPC, 1], F32, tag="psVb",
                                   name=f"psvb{m}")
                    for d in range(8):
                        nc.tensor.matmul(
                            psa[:], xv_t[d][:, m * 128:(m + 1) * 128],
                            wvP_t[d][:], start=(d == 0), stop=False)
                        nc.tensor.matmul(
                            psb[:], xv_t[d][:, m * 128:(m + 1) * 128],
                            wv64_t[d][:], start=(d == 0), stop=False)
                    # bias rides last so m0 does not wait on the bias DMA
                    nc.tensor.matmul(psa[:], ones_col[0:1, :],
                                     bvP_t[0:1, :], start=False, stop=True)
                    nc.tensor.matmul(psb[:], ones_col[0:1, :],
                                     bv64_t[0:1, :], start=False, stop=True)
                    nc.vector.tensor_copy(vh[m][:, :, 0:64], psa[:])
                    nc.vector.tensor_copy(vh[m][:, :, 64:65], psb[:])

            # xq reuses the V buffers (auto-synced on last V-proj reader)
            xq_t = load_x(xq_d, "q")

            # ------- phase 1b: K and Q projections, direct [dh, s] ---------
            # head-pair packing: stationary = [head 2p dh0-63 | head 2p+1
            # dh0-63] (128 cols), plus one 8-col straggler for dh64 of all
            # heads; evacuation DMA-shifts the upper half down to partition 0
            pevac = _xw_stack.enter_context(tc.tile_pool(name="pevac",
                                                         bufs=1))
            with tc.tile_pool(name="psKQ", bufs=2, space="PSUM") as psKQ:
                for pi, wP, w64, xt, bP, b64 in (
                        (1, wkP_t, wk64_t, xk_t, bkP_t, bk64_t),
                        (0, wqP_t, wq64_t, xq_t, bqP_t, bq64_t)):
                    ps8 = psKQ.tile([HPC, 4, QW], F32, tag="psKQ",
                                    name=f"ps64_{pi}")
                    for d in range(8):
                        for c in range(4):
                            nc.tensor.matmul(
                                ps8[:, c, :], w64[d][:],
                                xt[d][:, c * QW:(c + 1) * QW],
                                start=(d == 0), stop=(d == 7))
                    for c in range(4):
                        tmp8 = pevac.tile([HPC, QW], BF16, tag="tmp8",
                                          bufs=2, name=f"tmp8_{pi}_{c}")
                        nc.vector.tensor_scalar_add(
                            tmp8[:], ps8[:, c, :], b64[:, 0:1])
                        nc.gpsimd.dma_start(
                            qkT[64:65, pi, :, c * QW:(c + 1) * QW],
                            tmp8[:])
                    for p in range(4):
                        ps = psKQ.tile([128, 4, QW], F32, tag="psKQ",
                                       name=f"pskq{pi}_{p}")
                        for d in range(8):
                            for c in range(4):
                                nc.tensor.matmul(
                                    ps[:, c, :], wP[d][:, p, :],
                                    xt[d][:, c * QW:(c + 1) * QW],
                                    start=(d == 0), stop=(d == 7))
                        for c in range(4):
                            tmp = pevac.tile([128, QW], BF16, tag="tmpP",
                                             bufs=4, name=f"tmpP{pi}_{p}_{c}")
                            nc.vector.tensor_scalar_add(
                                tmp[:], ps[:, c, :], bP[:, p:p + 1])
                            nc.gpsimd.dma_start(
                                qkT[0:64, pi, 2 * p, c * QW:(c + 1) * QW],
                                tmp[0:64, :])
                            nc.gpsimd.dma_start(
                                qkT[0:64, pi, 2 * p + 1,
                                    c * QW:(c + 1) * QW],
                                tmp[64:128, :])
            _xw_stack.close()   # free x/w pools before attention phase

            # ---------------- phase 2+3: attention + out-proj --------------
            with tc.tile_pool(name="pm", bufs=3) as pm, \
                 tc.tile_pool(name="pp", bufs=4) as pp, \
                 tc.tile_pool(name="pc", bufs=1) as pc, \
                 tc.tile_pool(name="pwo", bufs=1) as pwo, \
                 tc.tile_pool(name="po", bufs=2) as po, \
                 tc.tile_pool(name="pt2", bufs=4) as pt2, \
                 tc.tile_pool(name="psS", bufs=2, space="PSUM") as psS, \
                 tc.tile_pool(name="psA", bufs=4, space="PSUM") as psA:

                # packed concatT: 128-row tiles covering rows 0..520
                ccp = [pc.tile([b - a, S], BF16, tag=f"ccp{i}",
                               name=f"ccp{i}")
                       for i, (a, b) in enumerate(PKT)]
                wop = []
                for i, (a, b) in enumerate(PKT):
                    w = pwo.tile([b - a, D], BF16, tag=f"wop{i}",
                                 name=f"wop{i}")
                    nc.sync.dma_start(w[:], wo_d[a:b, :])
                    wop.append(w)

                inv_sqrt = 1.0 / math.sqrt(float(DH))
                state = {}
                # AV rounds lag the score/exp stream by AV_LAG slots so the
                # next head's scores issue before this head's tail AV matmuls
                # (keeps the scalar engine's exp stream gapless); bgq drips
                # norm/out-proj PE work one small closure per round slot.
                pending = deque()
                bgq = deque()
                AV_LAG = 2

                def drain_slot(extra_bg=False):
                    if len(pending) > AV_LAG:
                        pending.popleft()()
                    if bgq:
                        bgq.popleft()()
                    if extra_bg and bgq:
                        bgq.popleft()()

                def attn_head(qb, h, mts, rsall, uovs):
                    ctx = {}
                    for r in range(NR):
                        ss = psS.tile([128, RKT, QW], F32, tag="psS",
                                      name=f"ss{qb}_{h}_{r}")
                        for jj in range(RKT):
                            j = r * RKT + jj
                            nc.tensor.matmul(
                                ss[:, jj, :],
                                qkT[0:DH, 1, h, j * 128:(j + 1) * 128],
                                qkT[0:DH, 0, h, qb * QW:(qb + 1) * QW],
                                start=True, stop=True)
                        pt = pp.tile([128, RKT, QW], BF16, tag="pT",
                                     name=f"pt{qb}_{h}_{r}")
                        nc.scalar.activation(
                            pt[:], ss[:],
                            mybir.ActivationFunctionType.Exp,
                            scale=inv_sqrt)
                        mt = mts[r // (NR // 2)]
                        rr = r % (NR // 2)
                        nc.vector.tensor_mul(
                            pt[:], pt[:], mt[:, rr * RKT:(rr + 1) * RKT, :])

                        def av(r=r, pt=pt, qb=qb, h=h, ctx=ctx):
                            if "ov" not in ctx:
                                ctx["ov"] = psA.tile(
                                    [128, QW], F32, tag="psA",
                                    name=f"ov{qb}_{h}")
                            ov = ctx["ov"]
                            for jj in range(RKT):
                                j = r * RKT + jj
                                nc.tensor.matmul(
                                    ov[0:DH + 1, :], vh[j][:, h, :],
                                    pt[:, jj, :],
                                    start=(j == 0), stop=(j == KT - 1))
                        pending.append(av)
                        drain_slot(extra_bg=(h >= 6))

                    def evac(qb=qb, h=h, ctx=ctx, rsall=rsall, uovs=uovs):
                        ov = ctx["ov"]
                        rs2 = pt2.tile([66, QW], F32, tag="rs2",
                                       name=f"rs2_{qb}_{h}", bufs=2)
                        nc.vector.tensor_copy(rs2[64:66, :], ov[64:66, :])
                        nc.sync.dma_start(
                            rsall[h // 4][h % 4:h % 4 + 1, :],
                            rs2[65:66, :])
                        uov = pt2.tile([DH, QW], BF16, tag="uov",
                                       name=f"uov{qb}_{h}", bufs=2 * HPC + 1)
                        nc.vector.tensor_copy(uov[:], ov[0:DH, :])
                        uovs.append(uov)
                    pending.append(evac)
                    # heads push 9 closures over 8 round slots; one extra pop
                    # here keeps the queue depth (and the AV lag) constant
                    if len(pending) > AV_LAG:
                        pending.popleft()()

                def norm_head_cl(qb, h):
                    def cl():
                        st = state[qb]
                        half = h // 4
                        key = f"rc{half}"
                        if key not in st:
                            rc = pt2.tile([4, QW], F32, tag="rcall",
                                          name=f"rcall{qb}_{half}", bufs=2)
                            nc.vector.reciprocal(
                                rc[:], st["rsall"][half][:])
                            # bf16 copy so the broadcast matmul below is not
                            # a pipeline-poisoning fp32 matmul
                            rcb = pt2.tile([4, QW], BF16, tag="rcb",
                                           name=f"rcb{qb}_{half}", bufs=2)
                            nc.vector.tensor_copy(rcb[:], rc[:])
                            st[key] = rcb
                        rbp = psA.tile([128, QW], F32, tag="psA",
                                       name=f"rbp{qb}_{h}")
                        nc.tensor.matmul(rbp[0:DH, :],
                                         sel8h[half][:, h * DH:(h + 1) * DH],
                                         st[key][:],
                                         start=True, stop=True)
                        cch = pt2.tile([DH, QW], BF16, tag="cch",
                                       name=f"cch{qb}_{h}", bufs=2)
                        nc.vector.tensor_mul(cch[:], rbp[0:DH, :],
                                             st["uovs"][h][:])
                        # pack into 128-row concatT tiles (DMA shifts rows)
                        r0 = h * DH
                        for i, (a, b) in enumerate(PKT):
                            lo, hi = max(r0, a), min(r0 + DH, b)
                            if lo < hi:
                                nc.gpsimd.dma_start(
                                    ccp[i][lo - a:hi - a,
                                           qb * QW:(qb + 1) * QW],
                                    cch[lo - r0:hi - r0, :])
                    return cl

                def outproj_cls(m):
                    ctx = {}

                    def mk_mm(n, i, a, b):
                        def cl():
                            if n not in ctx:
                                ctx[n] = psA.tile([128, QW], F32, tag="psA",
                                                  name=f"psop{m}_{n}")
                            nc.tensor.matmul(
                                ctx[n][:], ccp[i][:, m * 128:(m + 1) * 128],
                                wop[i][:, n * QW:(n + 1) * QW],
                                start=(i == 0), stop=(i == len(PKT) - 1))
                        return cl

                    def mk_copy(n):
                        def cl():
                            if "osb" not in ctx:
                                ctx["osb"] = po.tile([128, D], F32, tag="osb",
                                                     name=f"osb{m}")
                            nc.vector.tensor_copy(
                                ctx["osb"][:, n * QW:(n + 1) * QW],
                                ctx[n][:])
                        return cl

                    def mk_dma():
                        def cl():
                            nc.sync.dma_start(
                                out[m * 128:(m + 1) * 128, :], ctx["osb"][:])
                        return cl

                    cls = []
                    for n in range(2):
                        for i, (a, b) in enumerate(PKT):
                            cls.append(mk_mm(n, i, a, b))
                        cls.append(mk_copy(n))
                    cls.append(mk_dma())
                    return cls

                for qb in range(QB):
                    mts = []
                    for hf in range(2):
                        mt = pm.tile([128, KT // 2, QW], BF16, tag="mask",
                                     name=f"mask{qb}_{hf}")
                        nc.sync.dma_start(
                            mt[:], mh[qb, :, hf * (KT // 2) * QW:
                                      (hf + 1) * (KT // 2) * QW])
                        mts.append(mt)
                    rsall = [pt2.tile([4, QW], F32, tag=f"rsall{half}",
                                      name=f"rsall{qb}_{half}", bufs=2)
                             for half in range(2)]
                    uovs = []
                    state[qb] = {"rsall": rsall, "uovs": uovs}
                    for h in range(HPC):
                        attn_head(qb, h, mts, rsall, uovs)
                        if h == 4:
                            # evac(qb, h3) popped during h4 r1: heads 0-3 of
                            # this qb can normalize while it is still running
                            for hh in range(4):
                                bgq.append(norm_head_cl(qb, hh))
                        if qb > 0 and h == 0:
                            # evac(qb-1, h7) popped during h0 r1
                            for hh in range(4, HPC):
                                bgq.append(norm_head_cl(qb - 1, hh))
                            for m in range((qb - 1) * 4, qb * 4):
                                bgq.extend(outproj_cls(m))
                while pending:
                    pending.popleft()()
                for h in range(4, HPC):
                    bgq.append(norm_head_cl(QB - 1, h))
                for m in range((QB - 1) * 4, QB * 4):
                    bgq.extend(outproj_cls(m))
                while bgq:
                    bgq.popleft()()

    return nc


def _prep_inputs(q, k, v, mask, Wq, bqv, Wk, bkv, Wv, bvv, Wo):
    """Per-core input maps (numpy, host-side shard + cast)."""
    in_maps = []
    sel8 = np.zeros((HPC, HPC * DH), np.float32)
    for h in range(HPC):
        sel8[h, h * DH:(h + 1) * DH] = 1.0
    sel8 = sel8.astype(BF)
    mask_h = {}
    for b in range(B):
        mt = (mask[b, 0] != 0).astype(np.float32).T  # [k, q]
        m4 = mt.reshape(KT, 128, QB, QW).transpose(2, 1, 0, 3)
        mask_h[b] = np.ascontiguousarray(m4.reshape(QB, 128, KT * QW)).astype(BF)
    def pack_pair(Wt, bv_):
        # Wt [D, PC] -> pair-packed [D, 4, 128] + dh64 straggler [D, 8];
        # bias -> [128, 4] pair layout + [8, 1] straggler
        r = Wt.reshape(D, HPC, DH)
        # wP[:, p] = [head2p dh0-63 | head2p+1 dh0-63]
        wP = np.stack([np.concatenate([r[:, 2 * p, :64],
                                       r[:, 2 * p + 1, :64]], axis=1)
                       for p in range(4)], axis=1)
        w64 = r[:, :, 64]
        br = bv_.reshape(HPC, DH)
        bP = np.stack([np.concatenate([br[2 * p, :64], br[2 * p + 1, :64]])
                       for p in range(4)], axis=1)
        b64 = br[:, 64:65]
        return (np.ascontiguousarray(wP).astype(BF),
                np.ascontiguousarray(w64).astype(BF),
                np.ascontiguousarray(bP).astype(np.float32),
                np.ascontiguousarray(b64).astype(np.float32))

    for c in range(N_CORES):
        b, hh = c // 2, c % 2
        sl = slice(hh * PC, (hh + 1) * PC)
        wqP, wq64, bqP, bq64 = pack_pair(
            np.ascontiguousarray(Wq[sl, :].T), bqv[sl])
        wkP, wk64, bkP, bk64 = pack_pair(
            np.ascontiguousarray(Wk[sl, :].T), bkv[sl])
        rv = np.ascontiguousarray(Wv[sl, :].T).reshape(D, HPC, DH)
        bvr = bvv[sl].reshape(HPC, DH)
        in_maps.append({
            "xq": np.ascontiguousarray(q[b].T).astype(BF),
            "xk": np.ascontiguousarray(k[b].T).astype(BF),
            "xv": np.ascontiguousarray(v[b].T).astype(BF),
            "maskH": mask_h[b],
            "wqP": wqP, "wq64": wq64, "bqP": bqP, "bq64": bq64,
            "wkP": wkP, "wk64": wk64, "bkP": bkP, "bk64": bk64,
            "wvP": np.ascontiguousarray(
                rv[:, :, :64].reshape(D, 512)).astype(BF),
            "wv64": np.ascontiguousarray(rv[:, :, 64]).astype(BF),
            "bvP": bvr[:, :64].reshape(1, 512).astype(BF),
            "bv64": bvr[:, 64].reshape(1, HPC).astype(BF),
            "woT": np.ascontiguousarray(Wo[:, sl].T).astype(BF),
            "sel8": sel8,
        })
    return in_maps


def run_sharded(in_maps, **kwargs):
    if "nc" not in _BUILT:
        _BUILT["nc"] = _build_nc()
    return run_bass_kernel_spmd(_BUILT["nc"], in_maps,
                                core_ids=list(range(N_CORES)), **kwargs)


def kernel(q, k, v, mask, Wq, bq, Wk, bk, Wv, bv, Wo, bo):
    q = np.asarray(q, np.float32)
    k = np.asarray(k, np.float32)
    v = np.asarray(v, np.float32)
    mask = np.asarray(mask)
    in_maps = _prep_inputs(q, k, v, mask,
                           np.asarray(Wq, np.float32), np.asarray(bq, np.float32),
                           np.asarray(Wk, np.float32), np.asarray(bk, np.float32),
                           np.asarray(Wv, np.float32), np.asarray(bv, np.float32),
                           np.asarray(Wo, np.float32))
    res = run_sharded(in_maps)
    bo32 = np.asarray(bo, np.float32)
    out = np.empty((B, S, D), np.float32)
    for b in range(B):
        out[b] = res.results[2 * b]["out"] + res.results[2 * b + 1]["out"] + bo32
    return out
